# revision 34
# baseline (speedup 1.0000x reference)
"""Multi-head attention (B=2,S=2048,D=1024,H=16) on 8 TRN2 NeuronCores.

Sharding: core c handles head-PAIR c (heads 2c, 2c+1) of BOTH batches
(tensor parallel over heads). wq/wk/wv are split column-wise by pair,
wo row-wise. Each core computes partial output projections outT[b]
[D,S]; the host sums the 8 partials per batch, transposes, adds bo.

v2 schedule: the kernel is paced by the ScalarE exp stream (one
ACTIVATE of [128,1024] per (b,sq-block,key-tile)).  All projection /
output-projection matmuls are emitted as "filler" generator units that
are interleaved into the attention tile loop so the PE never starves
the exp stream, and the first exp issues as soon as the first k/q
512-col blocks are projected (~8us instead of ~46us).

Per-tile dataflow (device, "T" = [feature, seq] orientation):
  qT[b] = (wq_p^T @ xq_b^T) * 0.125       [128, S]
  kT[b] =  wk_p^T @ xk_b^T                [128, S]
  v[b]  =  xv_b @ wv_p                    [S, 128], + ones col/head
  per (b, sq-block, key-tile t):
    scoresT(hh) = kT_h[:,t]^T-stat @ qT_h    [128 sk, 512 sq] psum
      (hh=0 on PE rows 0-63, hh=1 rows 64-127 -> concurrent pair)
    pT = exp(scoresT)      one flat 1024-wide ACT call, no bias
    oT_ext(hh) += [v_h[t] | 1]^T-stat @ pT(hh)   [65, 512] psum
      (row 64 = softmax denominator via the ones column)
  norm: ots_ext = copy(otp) f32 (frees psum); recip row64 on DVE;
        ot = ots_ext[0:64] * bcast(recip)  (broadcast+mul on GpSimd)
  outT[b] += wo_p^T @ oT

Key-padding mask: only the (single) partial key tile's exp carries a
per-partition -30000 bias column; full tiles need no mask. Key tiles
beyond nblk_b are skipped entirely (identical loop bounds on every
core -> SPMD-safe).
"""

import sys

if "/opt/trn_rl_repo" not in sys.path:
    sys.path.insert(0, "/opt/trn_rl_repo")

from contextlib import ExitStack

import numpy as np
import ml_dtypes

from concourse import bass, bacc, mybir
from concourse import tile
from concourse.bass_utils import run_bass_kernel_spmd

BF16 = mybir.dt.bfloat16
F32 = mybir.dt.float32
npbf16 = ml_dtypes.bfloat16

B, S, D, H, DH = 2, 2048, 1024, 16, 64
NCORES = 8
PW = 2 * DH  # 128, head-pair width = per-core projection width
NKC = D // 128  # 8 contraction chunks for projections
NST = S // 128  # 16 key tiles max
SQB = 512
NSQB = S // SQB  # 4
NDT = D // 128  # 8 output row-tiles
SCALE = 1.0 / 8.0  # 1/sqrt(DH)

FILL_NS = 700.0  # PE-slack per attention tile-slot available for fillers


def build_nc(nblks, vrems, has_bqk, has_bv) -> bass.Bass:
    nc = bacc.Bacc()

    x_d = []
    for b in range(B):
        x_d.append(
            tuple(
                nc.declare_dram_parameter(f"x{n}t{b}", [D, S], BF16, isOutput=False)
                for n in "qkv"
            )
        )
    xview = [
        tuple(d.rearrange("(c p) s -> p c s", p=128) for d in x_d[b]) for b in range(B)
    ]
    wq_d = nc.declare_dram_parameter("wq", [128, NKC * PW], BF16, isOutput=False)
    wk_d = nc.declare_dram_parameter("wk", [128, NKC * PW], BF16, isOutput=False)
    wv_d = nc.declare_dram_parameter("wv", [128, NKC * PW], BF16, isOutput=False)
    wo_d = nc.declare_dram_parameter("wo", [128, D], BF16, isOutput=False)
    if has_bqk:
        bqk_d = nc.declare_dram_parameter("bqk", [128, 2], BF16, isOutput=False)
    if has_bv:
        bvb_d = nc.declare_dram_parameter("bvb", [128, 2 * DH], BF16, isOutput=False)
    need_mb = any(vrems[b] < 128 for b in range(B))
    if need_mb:
        mb_d = nc.declare_dram_parameter("mb", [128, B], F32, isOutput=False)
    out_d = nc.declare_dram_parameter("outt", [B * D, S], BF16, isOutput=True)
    out_v = out_d.rearrange("(x p) s -> p x s", p=128)  # [128, 16, S]

    Exp = mybir.ActivationFunctionType.Exp

    kmax = [nblks[b] * 128 for b in range(B)]
    nkb = [-(-kmax[b] // SQB) for b in range(B)]  # kT 512-col blocks

    with tile.TileContext(nc) as tc, ExitStack() as ctx:
        cpool = ctx.enter_context(tc.tile_pool(name="consts", bufs=1))
        xpool = ctx.enter_context(tc.tile_pool(name="xin", bufs=4))
        qkpool = ctx.enter_context(tc.tile_pool(name="qk", bufs=1))
        vpool = ctx.enter_context(tc.tile_pool(name="vsb", bufs=1))
        opool = ctx.enter_context(tc.tile_pool(name="osb", bufs=1))
        ptpool = ctx.enter_context(tc.tile_pool(name="ptp", bufs=4))
        nrmpool = ctx.enter_context(tc.tile_pool(name="nrm", bufs=2))
        outpool = ctx.enter_context(tc.tile_pool(name="outsb", bufs=2))
        pp = ctx.enter_context(tc.tile_pool(name="pp", bufs=2, space="PSUM"))
        sc = ctx.enter_context(tc.tile_pool(name="sc", bufs=2, space="PSUM"))
        otpp = ctx.enter_context(tc.tile_pool(name="otp", bufs=2, space="PSUM"))

        # ---- constant tiles ----
        wq_sb = cpool.tile([128, NKC * PW], BF16, tag="wq")
        wk_sb = cpool.tile([128, NKC * PW], BF16, tag="wk")
        wv_sb = cpool.tile([128, NKC * PW], BF16, tag="wv")
        wo_sb = cpool.tile([128, D], BF16, tag="wo")
        if has_bqk:
            bqk_sb = cpool.tile([128, 2], BF16, tag="bqk")
        if has_bv:
            bvb_sb = cpool.tile([128, 2 * DH], BF16, tag="bvb")
        if need_mb:
            mb_sb = cpool.tile([128, B], F32, tag="mb")

        qt_sb = qkpool.tile([128, B, S], BF16, tag="qt")
        kt_sb = qkpool.tile([128, B, S], BF16, tag="kt")
        # v with a TRAILING ones column per head: [sk-part, b, tile, head, dh+1]
        v_sb = vpool.tile([128, B, NST, 2, DH + 1], BF16, tag="v")
        ot_sb = opool.tile([128, B, S], BF16, tag="ot")

        xtiles = {}

        def xalloc(name):
            t = xpool.tile([128, NKC, S], BF16, tag="xt", name=name)
            xtiles[name] = t
            return t

        # ---- critical DMAs only (deps of the first scores + first AV);
        # everything else is issued from GpSimd gated behind kT block 0 so
        # the critical pieces get the full DMA bandwidth.
        nc.sync.dma_start(out=wk_sb[:], in_=wk_d[:])
        xk0 = xalloc("xk0")
        nc.sync.dma_start(
            out=xk0[:, :, 0 : min(512, kmax[0])],
            in_=xview[0][1][:, :, 0 : min(512, kmax[0])],
        )
        nc.sync.dma_start(out=wq_sb[:], in_=wq_d[:])
        xq0 = xalloc("xq0")
        nc.sync.dma_start(out=xq0[:, :, 0:SQB], in_=xview[0][0][:, :, 0:SQB])
        nc.sync.dma_start(out=wv_sb[:], in_=wv_d[:])
        if need_mb:
            nc.sync.dma_start(out=mb_sb[:], in_=mb_d[:])
        if has_bqk:
            nc.sync.dma_start(out=bqk_sb[:], in_=bqk_d[:])
        if has_bv:
            nc.sync.dma_start(out=bvb_sb[:], in_=bvb_d[:])
        xv0 = xalloc("xv0")
        nc.sync.dma_start(
            out=xv0[:, :, 0 : min(512, kmax[0])],
            in_=xview[0][2][:, :, 0 : min(512, kmax[0])],
        )

        # ones column of v (all tiles)
        nc.gpsimd.memset(v_sb[:, :, :, :, DH : DH + 1], 1.0)
        # PE warmup: data-independent junk matmuls keep HAM at K=8/8 while
        # the critical DMAs stream in, so the first projections run at 2.4GHz
        warm_sb = cpool.tile([128, 256], BF16, tag="warm")
        nc.vector.memset(warm_sb[:], 0.0)
        ones64 = cpool.tile([1, 64], F32, tag="ones64")
        nc.vector.memset(ones64[:], 1.0)
        for _ in range(34):
            wps = sc.tile([128, 2 * SQB], F32, tag="sc", name="warmps")
            nc.tensor.matmul(
                wps[:, 0:256], warm_sb[:, 0:128], warm_sb[:], start=True, stop=True
            )

        xk1 = xalloc("xk1")

        def emit_noncrit_dmas():
            for c0 in range(512, kmax[0], 512):
                c1 = min(c0 + 512, kmax[0])
                nc.sync.dma_start(out=xk0[:, :, c0:c1], in_=xview[0][1][:, :, c0:c1])
                nc.sync.dma_start(out=xv0[:, :, c0:c1], in_=xview[0][2][:, :, c0:c1])
            nc.sync.dma_start(
                out=xq0[:, :, SQB : 2 * SQB], in_=xview[0][0][:, :, SQB : 2 * SQB]
            )
            nc.sync.dma_start(
                out=xk1[:, :, 0 : kmax[1]], in_=xview[1][1][:, :, 0 : kmax[1]]
            )
            for sb in range(2, NSQB):
                nc.sync.dma_start(
                    out=xq0[:, :, sb * SQB : (sb + 1) * SQB],
                    in_=xview[0][0][:, :, sb * SQB : (sb + 1) * SQB],
                )
            nc.sync.dma_start(out=wo_sb[:], in_=wo_d[:])

        def load_xq1():
            t = xalloc("xq1")  # takes xk0's slot (kT0 done by then)
            for sb in range(NSQB):
                nc.sync.dma_start(
                    out=t[:, :, sb * SQB : (sb + 1) * SQB],
                    in_=xview[1][0][:, :, sb * SQB : (sb + 1) * SQB],
                )

        def load_xv1():
            t = xalloc("xv1")  # takes xq0's slot (qT0 done by then)
            for c0 in range(0, kmax[1], 768):
                c1 = min(c0 + 768, kmax[1])
                nc.sync.dma_start(out=t[:, :, c0:c1], in_=xview[1][2][:, :, c0:c1])

        # ---- projection emitters (psum-atomic units) ----
        def emit_qk_block(b, xt, sb, which, ncols=SQB):
            w_sb, dst, bcol = (
                (wq_sb, qt_sb, 0) if which == "q" else (wk_sb, kt_sb, 1)
            )
            c0 = sb * SQB
            ps = pp.tile([128, SQB], F32, tag="pp", name="psqk")
            for c in range(NKC):
                nc.tensor.matmul(
                    ps[:, 0:ncols],
                    w_sb[:, c * PW : (c + 1) * PW],
                    xt[:, c, c0 : c0 + ncols],
                    start=(c == 0),
                    stop=(c == NKC - 1),
                )
            if has_bqk:
                nc.vector.tensor_scalar_add(
                    dst[:, b, c0 : c0 + ncols],
                    ps[:, 0:ncols],
                    bqk_sb[:, bcol : bcol + 1],
                )
            else:
                nc.vector.tensor_copy(dst[:, b, c0 : c0 + ncols], ps[:, 0:ncols])

        def emit_v_tile(b, xt, t):
            psv = pp.tile([128, 2, DH], F32, tag="pp", name="psv")
            for c in range(NKC):
                nc.tensor.matmul(
                    psv[:],
                    xt[:, c, t * 128 : (t + 1) * 128],
                    wv_sb[:, c * PW : (c + 1) * PW],
                    start=(c == 0),
                    stop=(c == NKC - 1),
                )
            if has_bv:
                nc.vector.tensor_tensor(
                    v_sb[:, b, t, :, 0:DH],
                    psv[:],
                    bvb_sb[:].rearrange("p (hh dh) -> p hh dh", hh=2),
                    mybir.AluOpType.add,
                )
            else:
                nc.vector.tensor_copy(v_sb[:, b, t, :, 0:DH], psv[:])

        # ---- filler generator machinery ----
        # Generators yield (cost_ns) after each psum-atomic unit and update
        # prog[name]; the pacer pulls FIFO, drains force-pull by name.
        gens = []  # list of [name, iterator, done]
        gmap = {}
        prog = {}

        def gen_push(name, it):
            g = [name, it, False]
            gens.append(g)
            gmap[name] = g
            return g

        def _next(g):
            try:
                return next(g[1])
            except StopIteration:
                g[2] = True
                return 0.0

        def pull_one():
            while gens and gens[0][2]:
                gens.pop(0)
            if not gens:
                return 0.0
            return _next(gens[0])

        fill_credit = [0.0]

        def pull_fill(budget=FILL_NS):
            fill_credit[0] += budget
            while fill_credit[0] > 0.0:
                c = pull_one()
                if c == 0.0:
                    fill_credit[0] = min(fill_credit[0], FILL_NS)
                    break
                fill_credit[0] -= c

        def drain(name, upto=None):
            g = gmap.get(name)
            if g is None:
                return
            while not g[2] and (upto is None or prog.get(name, -1) < upto):
                _next(g)

        # generator bodies (each updates prog[name])
        def g_units(name, units):
            # units: list of (emit_fn, cost_ns)
            def it():
                for i, (fn, cost) in enumerate(units):
                    fn()
                    prog[name] = i
                    yield cost
            return it()

        def q_units(name, b, xtn, sbs):
            return g_units(
                name,
                [
                    (lambda sb=sb: emit_qk_block(b, xtiles[xtn], sb, "q"), 2000.0)
                    for sb in sbs
                ],
            )

        def k_units(name, b, xtn):
            return g_units(
                name,
                [
                    (
                        lambda blk=blk: emit_qk_block(
                            b, xtiles[xtn], blk, "k", min(SQB, kmax[b] - blk * SQB)
                        ),
                        2000.0,
                    )
                    for blk in range(nkb[b])
                ],
            )

        def v_units(name, b, xtn):
            return g_units(
                name,
                [
                    (lambda t=t: emit_v_tile(b, xtiles[xtn], t), 800.0)
                    for t in range(nblks[b])
                ],
            )

        def dma_unit(name, fn):
            return g_units(name, [(fn, 100.0)])

        def op_units(name, b, sqb, scalar_casts=False, split_dma=False):
            sq0 = sqb * SQB
            state = {}

            def mk(dt):
                def f():
                    if dt % 4 == 0:
                        state["osb"] = outpool.tile(
                            [128, 4, SQB], BF16, tag="outsb", name="osb"
                        )
                    pso = pp.tile([128, SQB], F32, tag="pp", name="pso")
                    nc.tensor.matmul(
                        pso[:],
                        wo_sb[:, dt * 128 : (dt + 1) * 128],
                        ot_sb[:, b, sq0 : sq0 + SQB],
                        start=True,
                        stop=True,
                    )
                    if scalar_casts and dt % 2 == 1:
                        nc.scalar.copy(state["osb"][:, dt % 4, :], pso[:])
                    else:
                        nc.vector.tensor_copy(state["osb"][:, dt % 4, :], pso[:])
                    if split_dma:
                        nc.sync.dma_start(
                            out=out_v[:, b * NDT + dt, sq0 : sq0 + SQB],
                            in_=state["osb"][:, dt % 4, :],
                        )
                    elif dt % 4 == 3:
                        nc.sync.dma_start(
                            out=out_v[
                                :, b * NDT + dt - 3 : b * NDT + dt + 1,
                                sq0 : sq0 + SQB,
                            ],
                            in_=state["osb"][:]
                        )
                return f

            return g_units(name, [(mk(dt), 300.0) for dt in range(NDT)])

        # ---- attention for one (batch, sq-block) ----
        def attention(b, sqb, jit_k=None, jit_v=None, need=(), last=False,
                      jit_fill=0.0):
            nblk = nblks[b]
            sq0 = sqb * SQB
            otp0 = otpp.tile([65, SQB], F32, tag="otp", name="otp0")
            otp1 = otpp.tile([65, SQB], F32, tag="otp", name="otp1")

            def emit_scores(t):
                if jit_k is not None:
                    # kT blocks 1.. are produced by gen jit_k (block idx-1)
                    blk = (t * 128) // SQB
                    if blk >= 1:
                        drain(jit_k, blk - 1)
                scp = sc.tile([128, 2 * SQB], F32, tag="sc", name="scp")
                nc.tensor.matmul(
                    scp[:, 0:SQB],
                    kt_sb[0:64, b, t * 128 : (t + 1) * 128],
                    qt_sb[0:64, b, sq0 : sq0 + SQB],
                    start=True,
                    stop=True,
                )
                nc.tensor.matmul(
                    scp[:, SQB : 2 * SQB],
                    kt_sb[64:128, b, t * 128 : (t + 1) * 128],
                    qt_sb[64:128, b, sq0 : sq0 + SQB],
                    start=True,
                    stop=True,
                )
                pt = ptpool.tile([128, 2 * SQB], BF16, tag="pt", name="pt")
                if t == nblk - 1 and vrems[b] < 128:
                    # key-padding mask: bias -30000 on rows >= vrem of the
                    # (only) partial tile; full tiles need no mask at all
                    nc.scalar.activation(
                        pt[:], scp[:], Exp, bias=mb_sb[:, b : b + 1]
                    )
                else:
                    nc.scalar.activation(pt[:], scp[:], Exp)
                return pt

            def emit_av(t, pt):
                for hh, otp in ((0, otp0), (1, otp1)):
                    nc.tensor.matmul(
                        otp[:],
                        v_sb[:, b, t, hh, :],
                        pt[:, hh * SQB : (hh + 1) * SQB],
                        start=(t == 0),
                        stop=(t == nblk - 1),
                    )

            if jit_v is not None:
                drain(jit_v, 0)
            needs = {}
            for ti, nm, upto in need:
                needs.setdefault(min(ti, nblk - 1), []).append((nm, upto))
            pts = emit_scores(0)
            for t in range(nblk):
                pt_next = emit_scores(t + 1) if t + 1 < nblk else None
                for nm, upto in needs.get(t, ()):
                    drain(nm, upto)
                if jit_v is not None:
                    drain(jit_v, t)  # v(t) must exist before AV(t)
                    if jit_fill and t < nblk - 2:
                        pull_fill(jit_fill)
                elif t < nblk - 2:
                    # no fills in the last 2 slots: the norm chain (which
                    # frees the otp psum banks) must lead the DVE queue
                    pull_fill()
                emit_av(t, pts)
                pts = pt_next

            # normalization: free otp banks fast via one f32 copy per head,
            # then recip (DVE) + broadcast/mul (GpSimd) off the PE path.
            exts = []
            for i, otp in enumerate((otp0, otp1)):
                ext = nrmpool.tile([65, SQB], F32, tag="ext", name="ext")
                if last and i == 1:
                    nc.scalar.copy(ext[:], otp[:])  # ScalarE idle after last exp
                else:
                    nc.vector.tensor_copy(ext[:], otp[:])
                exts.append(ext)
            for hh, ext in enumerate(exts):
                rs = nrmpool.tile([1, SQB], F32, tag="rs", name="rs")
                nc.vector.tensor_copy(rs[:], ext[64:65, :])
                recip = nrmpool.tile([1, SQB], F32, tag="recip", name="recip")
                nc.vector.reciprocal_approx_fast(recip[:], rs[:])
                bcast = nrmpool.tile([64, SQB], F32, tag="bcast", name="bcast")
                nc.gpsimd.partition_broadcast(bcast[:], recip[:])
                nc.vector.tensor_mul(
                    ot_sb[hh * 64 : hh * 64 + 64, b, sq0 : sq0 + SQB],
                    ext[0:64, :],
                    bcast[:],
                )

        # =================== schedule ===================
        # preamble: kT(b0) block 0 + qT(b0) sqb0 -> first scores ASAP
        emit_qk_block(0, xk0, 0, "k", min(SQB, kmax[0]))
        emit_noncrit_dmas()
        emit_qk_block(0, xq0, 0, "q")

        gen_push("k0rest", g_units(
            "k0rest",
            [
                (
                    lambda blk=blk: emit_qk_block(
                        0, xtiles["xk0"], blk, "k", min(SQB, kmax[0] - blk * SQB)
                    ),
                    2000.0,
                )
                for blk in range(1, nkb[0])
            ],
        ))
        gen_push("v0", v_units("v0", 0, "xv0"))
        gen_push("q0s1", q_units("q0s1", 0, "xq0", [1]))
        gen_push("q0s2", q_units("q0s2", 0, "xq0", [2]))
        gen_push("q0s3", q_units("q0s3", 0, "xq0", [3]))
        gen_push("k1", k_units("k1", 1, "xk1"))
        gen_push("xv1dma", dma_unit("xv1dma", load_xv1))
        gen_push("v1", v_units("v1", 1, "xv1"))
        gen_push("q1s0", q_units("q1s0", 1, "xq1", [0]))

        nb0, nb1 = nblks
        attention(0, 0, jit_k="k0rest", jit_v="v0",
                  need=[(nb0 - 2, "q0s1", None)])
        # xq1 reuses xk0's buffer slot; all xk0 reads (kT0) are emitted by now
        load_xq1()
        drain("q0s1")
        gen_push("op00", op_units("op00", 0, 0))

        attention(0, 1, need=[(nb0 - 2, "q0s2", None)])
        drain("q0s2")
        gen_push("op01", op_units("op01", 0, 1))

        attention(0, 2, need=[(nb0 - 2, "q0s3", None)])
        drain("q0s3")
        gen_push("op02", op_units("op02", 0, 2))

        gen_push("q1rest", q_units("q1rest", 1, "xq1", [1, 2, 3]))
        attention(0, 3, need=[(2, "k1", None), (nb0 - 1, "q1s0", None)])
        gen_push("op03", op_units("op03", 0, 3))

        drain("k1")
        drain("q1s0")
        attention(1, 0, jit_v="v1", jit_fill=400.0,
                  need=[(nb1 - 2, "q1rest", 0)])
        gen_push("op10", op_units("op10", 1, 0))

        drain("q1rest", upto=0)
        attention(1, 1, need=[(nb1 - 2, "q1rest", 1)])
        gen_push("op11", op_units("op11", 1, 1))

        drain("q1rest", upto=1)
        attention(1, 2, need=[(nb1 - 2, "q1rest", None)])
        gen_push("op12", op_units("op12", 1, 2))

        drain("q1rest")
        attention(1, 3, last=True)
        for _ in range(40):  # keep PE warm through the final norm chain
            wps = sc.tile([128, 2 * SQB], F32, tag="sc", name="warmps")
            nc.tensor.matmul(
                wps[:, 0:256], warm_sb[:, 0:128], warm_sb[:], start=True, stop=True
            )
        gen_push("op13", op_units("op13", 1, 3, scalar_casts=True, split_dma=True))

        while gens:
            pull_one()

    nc.compile()
    return nc


def _chunk_rows(w: np.ndarray, nchunk: int) -> np.ndarray:
    """[nchunk*128, C] -> [128, nchunk*C] with chunk-major columns."""
    c = w.shape[1]
    return np.ascontiguousarray(
        w.reshape(nchunk, 128, c).transpose(1, 0, 2).reshape(128, nchunk * c)
    )


def make_inmaps(inputs: dict):
    xq = np.asarray(inputs["xq"], np.float32)
    xk = np.asarray(inputs["xk"], np.float32)
    xv = np.asarray(inputs["xv"], np.float32)
    wq = np.asarray(inputs["wq"], np.float32)
    bq = np.asarray(inputs["bq"], np.float32)
    wk = np.asarray(inputs["wk"], np.float32)
    bk = np.asarray(inputs["bk"], np.float32)
    wv = np.asarray(inputs["wv"], np.float32)
    bv = np.asarray(inputs["bv"], np.float32)
    wo = np.asarray(inputs["wo"], np.float32)
    valid_lens = np.asarray(inputs["valid_lens"], np.int64)

    nblks = tuple(
        int(min(NST, max(1, -(-int(valid_lens[b]) // 128)))) for b in range(B)
    )
    vrems = tuple(
        min(128, int(valid_lens[b]) - (nblks[b] - 1) * 128) for b in range(B)
    )
    has_bqk = bool(np.any(bq != 0) or np.any(bk != 0))
    has_bv = bool(np.any(bv != 0))

    # shared per-batch transposed activations (bf16)
    xts = {}
    for b in range(B):
        for n, a in (("q", xq), ("k", xk), ("v", xv)):
            xts[f"x{n}t{b}"] = np.ascontiguousarray(a[b].T).astype(npbf16)

    need_mb = any(vrems[b] < 128 for b in range(B))
    if need_mb:
        mb = np.zeros((128, B), np.float32)
        for b in range(B):
            mb[vrems[b] :, b] = -30000.0

    in_maps = []
    for c in range(NCORES):
        sl = slice(c * PW, (c + 1) * PW)
        m = {
            **xts,
            "wq": _chunk_rows(wq[:, sl] * SCALE, NKC).astype(npbf16),
            "wk": _chunk_rows(wk[:, sl], NKC).astype(npbf16),
            "wv": _chunk_rows(wv[:, sl], NKC).astype(npbf16),
            "wo": np.ascontiguousarray(wo[sl, :]).astype(npbf16),
        }
        if need_mb:
            m["mb"] = mb
        if has_bqk:
            m["bqk"] = np.ascontiguousarray(
                np.stack([bq[sl] * SCALE, bk[sl]], axis=1)
            ).astype(npbf16)
        if has_bv:
            m["bvb"] = np.ascontiguousarray(
                np.broadcast_to(bv[sl][None, :], (128, 2 * DH))
            ).astype(npbf16)
        in_maps.append(m)
    return in_maps, nblks, vrems, has_bqk, has_bv


def assemble(results, inputs) -> np.ndarray:
    bo = np.asarray(inputs["bo"], np.float32)
    out = np.zeros((B, S, D), np.float32)
    for c in range(NCORES):
        part = np.asarray(results[c]["outt"], np.float32).reshape(B, D, S)
        for b in range(B):
            out[b] += part[b].T
    out += bo[None, None, :]
    return out


def kernel(**inputs) -> np.ndarray:
    in_maps, nblks, vrems, has_bqk, has_bv = make_inmaps(inputs)
    nc = build_nc(nblks, vrems, has_bqk, has_bv)
    res = run_bass_kernel_spmd(nc, in_maps, core_ids=list(range(NCORES)))
    return assemble(res.results, inputs)


if __name__ == "__main__":
    import reference

    inputs = reference.setup_inputs()
    out = kernel(**{k: np.asarray(v) for k, v in inputs.items()})
    exp = np.asarray(reference.reference(**inputs))
    err = np.linalg.norm(out - exp) / np.linalg.norm(exp)
    print("Relative error:", err)


# revision 35
# speedup vs baseline: 1.0197x; 1.0197x over previous
"""Multi-head attention (B=2,S=2048,D=1024,H=16) on 8 TRN2 NeuronCores.

Sharding: core c handles head-PAIR c (heads 2c, 2c+1) of BOTH batches
(tensor parallel over heads). wq/wk/wv are split column-wise by pair,
wo row-wise. Each core computes partial output projections outT[b]
[D,S]; the host sums the 8 partials per batch, transposes, adds bo.

v2 schedule: the kernel is paced by the ScalarE exp stream (one
ACTIVATE of [128,1024] per (b,sq-block,key-tile)).  All projection /
output-projection matmuls are emitted as "filler" generator units that
are interleaved into the attention tile loop so the PE never starves
the exp stream, and the first exp issues as soon as the first k/q
512-col blocks are projected (~8us instead of ~46us).

Per-tile dataflow (device, "T" = [feature, seq] orientation):
  qT[b] = (wq_p^T @ xq_b^T) * 0.125       [128, S]
  kT[b] =  wk_p^T @ xk_b^T                [128, S]
  v[b]  =  xv_b @ wv_p                    [S, 128], + ones col/head
  per (b, sq-block, key-tile t):
    scoresT(hh) = kT_h[:,t]^T-stat @ qT_h    [128 sk, 512 sq] psum
      (hh=0 on PE rows 0-63, hh=1 rows 64-127 -> concurrent pair)
    pT = exp(scoresT)      one flat 1024-wide ACT call, no bias
    oT_ext(hh) += [v_h[t] | 1]^T-stat @ pT(hh)   [65, 512] psum
      (row 64 = softmax denominator via the ones column)
  norm: ots_ext = copy(otp) f32 (frees psum); recip row64 on DVE;
        ot = ots_ext[0:64] * bcast(recip)  (broadcast+mul on GpSimd)
  outT[b] += wo_p^T @ oT

Key-padding mask: only the (single) partial key tile's exp carries a
per-partition -30000 bias column; full tiles need no mask. Key tiles
beyond nblk_b are skipped entirely (identical loop bounds on every
core -> SPMD-safe).
"""

import sys

if "/opt/trn_rl_repo" not in sys.path:
    sys.path.insert(0, "/opt/trn_rl_repo")

from contextlib import ExitStack

import numpy as np
import ml_dtypes

from concourse import bass, bacc, mybir
from concourse import tile
from concourse.bass_utils import run_bass_kernel_spmd

BF16 = mybir.dt.bfloat16
F32 = mybir.dt.float32
npbf16 = ml_dtypes.bfloat16

B, S, D, H, DH = 2, 2048, 1024, 16, 64
NCORES = 8
PW = 2 * DH  # 128, head-pair width = per-core projection width
NKC = D // 128  # 8 contraction chunks for projections
NST = S // 128  # 16 key tiles max
SQB = 512
NSQB = S // SQB  # 4
NDT = D // 128  # 8 output row-tiles
SCALE = 1.0 / 8.0  # 1/sqrt(DH)

FILL_NS = 700.0  # PE-slack per attention tile-slot available for fillers


def build_nc(nblks, vrems, has_bqk, has_bv) -> bass.Bass:
    nc = bacc.Bacc()

    x_d = []
    for b in range(B):
        x_d.append(
            tuple(
                nc.declare_dram_parameter(f"x{n}t{b}", [D, S], BF16, isOutput=False)
                for n in "qkv"
            )
        )
    xview = [
        tuple(d.rearrange("(c p) s -> p c s", p=128) for d in x_d[b]) for b in range(B)
    ]
    wq_d = nc.declare_dram_parameter("wq", [128, NKC * PW], BF16, isOutput=False)
    wk_d = nc.declare_dram_parameter("wk", [128, NKC * PW], BF16, isOutput=False)
    wv_d = nc.declare_dram_parameter("wv", [128, NKC * PW], BF16, isOutput=False)
    wo_d = nc.declare_dram_parameter("wo", [128, D], BF16, isOutput=False)
    if has_bqk:
        bqk_d = nc.declare_dram_parameter("bqk", [128, 2], BF16, isOutput=False)
    if has_bv:
        bvb_d = nc.declare_dram_parameter("bvb", [128, 2 * DH], BF16, isOutput=False)
    need_mb = any(vrems[b] < 128 for b in range(B))
    if need_mb:
        mb_d = nc.declare_dram_parameter("mb", [128, B], F32, isOutput=False)
    out_d = nc.declare_dram_parameter("outt", [B * D, S], BF16, isOutput=True)
    out_v = out_d.rearrange("(x p) s -> p x s", p=128)  # [128, 16, S]

    Exp = mybir.ActivationFunctionType.Exp

    kmax = [nblks[b] * 128 for b in range(B)]
    nkb = [-(-kmax[b] // SQB) for b in range(B)]  # kT 512-col blocks

    with tile.TileContext(nc) as tc, ExitStack() as ctx:
        cpool = ctx.enter_context(tc.tile_pool(name="consts", bufs=1))
        xpool = ctx.enter_context(tc.tile_pool(name="xin", bufs=4))
        qkpool = ctx.enter_context(tc.tile_pool(name="qk", bufs=1))
        vpool = ctx.enter_context(tc.tile_pool(name="vsb", bufs=1))
        opool = ctx.enter_context(tc.tile_pool(name="osb", bufs=1))
        ptpool = ctx.enter_context(tc.tile_pool(name="ptp", bufs=3))
        nrmpool = ctx.enter_context(tc.tile_pool(name="nrm", bufs=2))
        outpool = ctx.enter_context(tc.tile_pool(name="outsb", bufs=2))
        pp = ctx.enter_context(tc.tile_pool(name="pp", bufs=2, space="PSUM"))
        sc = ctx.enter_context(tc.tile_pool(name="sc", bufs=2, space="PSUM"))
        otpp = ctx.enter_context(tc.tile_pool(name="otp", bufs=2, space="PSUM"))

        # ---- constant tiles ----
        wq_sb = cpool.tile([128, NKC * PW], BF16, tag="wq")
        wk_sb = cpool.tile([128, NKC * PW], BF16, tag="wk")
        wv_sb = cpool.tile([128, NKC * PW], BF16, tag="wv")
        wo_sb = cpool.tile([128, D], BF16, tag="wo")
        if has_bqk:
            bqk_sb = cpool.tile([128, 2], BF16, tag="bqk")
        if has_bv:
            bvb_sb = cpool.tile([128, 2 * DH], BF16, tag="bvb")
        if need_mb:
            mb_sb = cpool.tile([128, B], F32, tag="mb")

        qt_sb = qkpool.tile([128, B, S], BF16, tag="qt")
        kt_sb = qkpool.tile([128, B, S], BF16, tag="kt")
        # v with a TRAILING ones column per head: [sk-part, b, tile, head, dh+1]
        v_sb = vpool.tile([128, B, NST, 2, DH + 1], BF16, tag="v")
        ot_sb = opool.tile([128, B, S], BF16, tag="ot")

        xtiles = {}

        def xalloc(name):
            t = xpool.tile([128, NKC, S], BF16, tag="xt", name=name)
            xtiles[name] = t
            return t

        # ---- critical DMAs only (deps of the first scores + first AV);
        # everything else is issued from GpSimd gated behind kT block 0 so
        # the critical pieces get the full DMA bandwidth.
        nc.sync.dma_start(out=wk_sb[:], in_=wk_d[:])
        xk0 = xalloc("xk0")
        nc.sync.dma_start(
            out=xk0[:, :, 0 : min(512, kmax[0])],
            in_=xview[0][1][:, :, 0 : min(512, kmax[0])],
        )
        nc.sync.dma_start(out=wq_sb[:], in_=wq_d[:])
        xq0 = xalloc("xq0")
        nc.sync.dma_start(out=xq0[:, :, 0:SQB], in_=xview[0][0][:, :, 0:SQB])
        nc.sync.dma_start(out=wv_sb[:], in_=wv_d[:])
        if need_mb:
            nc.sync.dma_start(out=mb_sb[:], in_=mb_d[:])
        if has_bqk:
            nc.sync.dma_start(out=bqk_sb[:], in_=bqk_d[:])
        if has_bv:
            nc.sync.dma_start(out=bvb_sb[:], in_=bvb_d[:])
        xv0 = xalloc("xv0")
        nc.sync.dma_start(
            out=xv0[:, :, 0 : min(512, kmax[0])],
            in_=xview[0][2][:, :, 0 : min(512, kmax[0])],
        )

        # ones column of v (all tiles)
        nc.gpsimd.memset(v_sb[:, :, :, :, DH : DH + 1], 1.0)
        # PE warmup: data-independent junk matmuls keep HAM at K=8/8 while
        # the critical DMAs stream in, so the first projections run at 2.4GHz
        warm_sb = cpool.tile([128, 256], BF16, tag="warm")
        nc.vector.memset(warm_sb[:], 0.0)
        ones64 = cpool.tile([1, 64], F32, tag="ones64")
        nc.vector.memset(ones64[:], 1.0)
        for _ in range(34):
            wps = sc.tile([128, 2 * SQB], F32, tag="sc", name="warmps")
            nc.tensor.matmul(
                wps[:, 0:256], warm_sb[:, 0:128], warm_sb[:], start=True, stop=True
            )

        xk1 = xalloc("xk1")

        def emit_noncrit_dmas():
            for c0 in range(512, kmax[0], 512):
                c1 = min(c0 + 512, kmax[0])
                nc.sync.dma_start(out=xk0[:, :, c0:c1], in_=xview[0][1][:, :, c0:c1])
                nc.sync.dma_start(out=xv0[:, :, c0:c1], in_=xview[0][2][:, :, c0:c1])
            nc.sync.dma_start(
                out=xq0[:, :, SQB : 2 * SQB], in_=xview[0][0][:, :, SQB : 2 * SQB]
            )
            nc.sync.dma_start(
                out=xk1[:, :, 0 : kmax[1]], in_=xview[1][1][:, :, 0 : kmax[1]]
            )
            for sb in range(2, NSQB):
                nc.sync.dma_start(
                    out=xq0[:, :, sb * SQB : (sb + 1) * SQB],
                    in_=xview[0][0][:, :, sb * SQB : (sb + 1) * SQB],
                )
            nc.sync.dma_start(out=wo_sb[:], in_=wo_d[:])

        def load_xq1():
            t = xalloc("xq1")  # takes xk0's slot (kT0 done by then)
            for sb in range(NSQB):
                nc.sync.dma_start(
                    out=t[:, :, sb * SQB : (sb + 1) * SQB],
                    in_=xview[1][0][:, :, sb * SQB : (sb + 1) * SQB],
                )

        def load_xv1():
            t = xalloc("xv1")  # takes xq0's slot (qT0 done by then)
            for c0 in range(0, kmax[1], 768):
                c1 = min(c0 + 768, kmax[1])
                nc.sync.dma_start(out=t[:, :, c0:c1], in_=xview[1][2][:, :, c0:c1])

        # ---- projection emitters (psum-atomic units) ----
        def emit_qk_block(b, xt, sb, which, ncols=SQB):
            w_sb, dst, bcol = (
                (wq_sb, qt_sb, 0) if which == "q" else (wk_sb, kt_sb, 1)
            )
            c0 = sb * SQB
            ps = pp.tile([128, SQB], F32, tag="pp", name="psqk")
            for c in range(NKC):
                nc.tensor.matmul(
                    ps[:, 0:ncols],
                    w_sb[:, c * PW : (c + 1) * PW],
                    xt[:, c, c0 : c0 + ncols],
                    start=(c == 0),
                    stop=(c == NKC - 1),
                )
            if has_bqk:
                nc.vector.tensor_scalar_add(
                    dst[:, b, c0 : c0 + ncols],
                    ps[:, 0:ncols],
                    bqk_sb[:, bcol : bcol + 1],
                )
            else:
                nc.vector.tensor_copy(dst[:, b, c0 : c0 + ncols], ps[:, 0:ncols])

        def emit_v_tile(b, xt, t):
            psv = pp.tile([128, 2, DH], F32, tag="pp", name="psv")
            for c in range(NKC):
                nc.tensor.matmul(
                    psv[:],
                    xt[:, c, t * 128 : (t + 1) * 128],
                    wv_sb[:, c * PW : (c + 1) * PW],
                    start=(c == 0),
                    stop=(c == NKC - 1),
                )
            if has_bv:
                nc.vector.tensor_tensor(
                    v_sb[:, b, t, :, 0:DH],
                    psv[:],
                    bvb_sb[:].rearrange("p (hh dh) -> p hh dh", hh=2),
                    mybir.AluOpType.add,
                )
            else:
                nc.vector.tensor_copy(v_sb[:, b, t, :, 0:DH], psv[:])

        # ---- filler generator machinery ----
        # Generators yield (cost_ns) after each psum-atomic unit and update
        # prog[name]; the pacer pulls FIFO, drains force-pull by name.
        gens = []  # list of [name, iterator, done]
        gmap = {}
        prog = {}

        def gen_push(name, it):
            g = [name, it, False]
            gens.append(g)
            gmap[name] = g
            return g

        def _next(g):
            try:
                return next(g[1])
            except StopIteration:
                g[2] = True
                return 0.0

        def pull_one():
            while gens and gens[0][2]:
                gens.pop(0)
            if not gens:
                return 0.0
            return _next(gens[0])

        fill_credit = [0.0]

        def pull_fill(budget=FILL_NS):
            fill_credit[0] += budget
            while fill_credit[0] > 0.0:
                c = pull_one()
                if c == 0.0:
                    fill_credit[0] = min(fill_credit[0], FILL_NS)
                    break
                fill_credit[0] -= c

        def drain(name, upto=None):
            g = gmap.get(name)
            if g is None:
                return
            while not g[2] and (upto is None or prog.get(name, -1) < upto):
                _next(g)

        # generator bodies (each updates prog[name])
        def g_units(name, units):
            # units: list of (emit_fn, cost_ns)
            def it():
                for i, (fn, cost) in enumerate(units):
                    fn()
                    prog[name] = i
                    yield cost
            return it()

        def q_units(name, b, xtn, sbs):
            return g_units(
                name,
                [
                    (lambda sb=sb: emit_qk_block(b, xtiles[xtn], sb, "q"), 2000.0)
                    for sb in sbs
                ],
            )

        def k_units(name, b, xtn):
            return g_units(
                name,
                [
                    (
                        lambda blk=blk: emit_qk_block(
                            b, xtiles[xtn], blk, "k", min(SQB, kmax[b] - blk * SQB)
                        ),
                        2000.0,
                    )
                    for blk in range(nkb[b])
                ],
            )

        def v_units(name, b, xtn):
            return g_units(
                name,
                [
                    (lambda t=t: emit_v_tile(b, xtiles[xtn], t), 800.0)
                    for t in range(nblks[b])
                ],
            )

        def dma_unit(name, fn):
            return g_units(name, [(fn, 100.0)])

        def op_units(name, b, sqb, scalar_casts=False, split_dma=False):
            sq0 = sqb * SQB
            state = {}

            def mk(dt):
                def f():
                    if dt == 0:
                        state["osb"] = outpool.tile(
                            [128, NDT, SQB], BF16, tag="outsb", name="osb"
                        )
                    pso = pp.tile([128, SQB], F32, tag="pp", name="pso")
                    nc.tensor.matmul(
                        pso[:],
                        wo_sb[:, dt * 128 : (dt + 1) * 128],
                        ot_sb[:, b, sq0 : sq0 + SQB],
                        start=True,
                        stop=True,
                    )
                    if scalar_casts and dt % 2 == 1:
                        nc.scalar.copy(state["osb"][:, dt, :], pso[:])
                    else:
                        nc.vector.tensor_copy(state["osb"][:, dt, :], pso[:])
                    if split_dma:
                        nc.sync.dma_start(
                            out=out_v[:, b * NDT + dt, sq0 : sq0 + SQB],
                            in_=state["osb"][:, dt, :],
                        )
                    elif dt == NDT - 1:
                        nc.sync.dma_start(
                            out=out_v[:, b * NDT : (b + 1) * NDT, sq0 : sq0 + SQB],
                            in_=state["osb"][:]
                        )
                return f

            return g_units(name, [(mk(dt), 300.0) for dt in range(NDT)])

        # ---- attention for one (batch, sq-block) ----
        def attention(b, sqb, jit_k=None, jit_v=None, need=(), last=False):
            nblk = nblks[b]
            sq0 = sqb * SQB
            otp0 = otpp.tile([65, SQB], F32, tag="otp", name="otp0")
            otp1 = otpp.tile([65, SQB], F32, tag="otp", name="otp1")

            def emit_scores(t):
                if jit_k is not None:
                    # kT blocks 1.. are produced by gen jit_k (block idx-1)
                    blk = (t * 128) // SQB
                    if blk >= 1:
                        drain(jit_k, blk - 1)
                scp = sc.tile([128, 2 * SQB], F32, tag="sc", name="scp")
                nc.tensor.matmul(
                    scp[:, 0:SQB],
                    kt_sb[0:64, b, t * 128 : (t + 1) * 128],
                    qt_sb[0:64, b, sq0 : sq0 + SQB],
                    start=True,
                    stop=True,
                )
                nc.tensor.matmul(
                    scp[:, SQB : 2 * SQB],
                    kt_sb[64:128, b, t * 128 : (t + 1) * 128],
                    qt_sb[64:128, b, sq0 : sq0 + SQB],
                    start=True,
                    stop=True,
                )
                pt = ptpool.tile([128, 2 * SQB], BF16, tag="pt", name="pt")
                if t == nblk - 1 and vrems[b] < 128:
                    # key-padding mask: bias -30000 on rows >= vrem of the
                    # (only) partial tile; full tiles need no mask at all
                    nc.scalar.activation(
                        pt[:], scp[:], Exp, bias=mb_sb[:, b : b + 1]
                    )
                else:
                    nc.scalar.activation(pt[:], scp[:], Exp)
                return pt

            def emit_av(t, pt):
                for hh, otp in ((0, otp0), (1, otp1)):
                    nc.tensor.matmul(
                        otp[:],
                        v_sb[:, b, t, hh, :],
                        pt[:, hh * SQB : (hh + 1) * SQB],
                        start=(t == 0),
                        stop=(t == nblk - 1),
                    )

            if jit_v is not None:
                drain(jit_v, 0)
            needs = {}
            for ti, nm, upto in need:
                needs.setdefault(min(ti, nblk - 1), []).append((nm, upto))
            pts = emit_scores(0)
            for t in range(nblk):
                pt_next = emit_scores(t + 1) if t + 1 < nblk else None
                for nm, upto in needs.get(t, ()):
                    drain(nm, upto)
                if jit_v is not None:
                    drain(jit_v, t)  # v(t) must exist before AV(t)
                elif t < nblk - 2:
                    # no fills in the last 2 slots: the norm chain (which
                    # frees the otp psum banks) must lead the DVE queue
                    pull_fill()
                emit_av(t, pts)
                pts = pt_next

            # normalization: free otp banks fast via one f32 copy per head,
            # then recip (DVE) + broadcast/mul (GpSimd) off the PE path.
            exts = []
            for i, otp in enumerate((otp0, otp1)):
                ext = nrmpool.tile([65, SQB], F32, tag="ext", name="ext")
                if last and i == 1:
                    nc.scalar.copy(ext[:], otp[:])  # ScalarE idle after last exp
                else:
                    nc.vector.tensor_copy(ext[:], otp[:])
                exts.append(ext)
            for hh, ext in enumerate(exts):
                rs = nrmpool.tile([1, SQB], F32, tag="rs", name="rs")
                nc.vector.tensor_copy(rs[:], ext[64:65, :])
                recip = nrmpool.tile([1, SQB], F32, tag="recip", name="recip")
                nc.vector.reciprocal_approx_fast(recip[:], rs[:])
                bcast = nrmpool.tile([64, SQB], F32, tag="bcast", name="bcast")
                nc.gpsimd.partition_broadcast(bcast[:], recip[:])
                nc.vector.tensor_mul(
                    ot_sb[hh * 64 : hh * 64 + 64, b, sq0 : sq0 + SQB],
                    ext[0:64, :],
                    bcast[:],
                )

        # =================== schedule ===================
        # preamble: kT(b0) block 0 + qT(b0) sqb0 -> first scores ASAP
        emit_qk_block(0, xk0, 0, "k", min(SQB, kmax[0]))
        emit_noncrit_dmas()
        emit_qk_block(0, xq0, 0, "q")

        gen_push("k0rest", g_units(
            "k0rest",
            [
                (
                    lambda blk=blk: emit_qk_block(
                        0, xtiles["xk0"], blk, "k", min(SQB, kmax[0] - blk * SQB)
                    ),
                    2000.0,
                )
                for blk in range(1, nkb[0])
            ],
        ))
        gen_push("v0", v_units("v0", 0, "xv0"))
        gen_push("q0s1", q_units("q0s1", 0, "xq0", [1]))
        gen_push("q0s2", q_units("q0s2", 0, "xq0", [2]))
        gen_push("q0s3", q_units("q0s3", 0, "xq0", [3]))
        gen_push("k1", k_units("k1", 1, "xk1"))
        gen_push("xv1dma", dma_unit("xv1dma", load_xv1))
        gen_push("v1", v_units("v1", 1, "xv1"))
        gen_push("q1s0", q_units("q1s0", 1, "xq1", [0]))

        nb0, nb1 = nblks
        attention(0, 0, jit_k="k0rest", jit_v="v0",
                  need=[(nb0 - 2, "q0s1", None)])
        # xq1 reuses xk0's buffer slot; all xk0 reads (kT0) are emitted by now
        load_xq1()
        drain("q0s1")
        gen_push("op00", op_units("op00", 0, 0))

        attention(0, 1, need=[(nb0 - 2, "q0s2", None)])
        drain("q0s2")
        gen_push("op01", op_units("op01", 0, 1))

        attention(0, 2, need=[(nb0 - 2, "q0s3", None)])
        drain("q0s3")
        gen_push("op02", op_units("op02", 0, 2))

        gen_push("q1rest", q_units("q1rest", 1, "xq1", [1, 2, 3]))
        attention(0, 3, need=[(2, "k1", None), (nb0 - 3, "v1", None),
                              (nb0 - 1, "q1s0", None)])
        gen_push("op03", op_units("op03", 0, 3))

        drain("k1")
        drain("v1")
        drain("q1s0")
        attention(1, 0, need=[(nb1 - 2, "q1rest", 0)])
        gen_push("op10", op_units("op10", 1, 0))

        drain("q1rest", upto=0)
        attention(1, 1, need=[(nb1 - 2, "q1rest", 1)])
        gen_push("op11", op_units("op11", 1, 1))

        drain("q1rest", upto=1)
        attention(1, 2, need=[(nb1 - 2, "q1rest", None)])
        gen_push("op12", op_units("op12", 1, 2))

        drain("q1rest")
        attention(1, 3, last=True)
        for _ in range(12):  # keep PE warm through the final norm chain
            wps = sc.tile([128, 2 * SQB], F32, tag="sc", name="warmps")
            nc.tensor.matmul(
                wps[:, 0:256], warm_sb[:, 0:128], warm_sb[:], start=True, stop=True
            )
        gen_push("op13", op_units("op13", 1, 3, scalar_casts=True, split_dma=True))

        while gens:
            pull_one()

    nc.compile()
    return nc


def _chunk_rows(w: np.ndarray, nchunk: int) -> np.ndarray:
    """[nchunk*128, C] -> [128, nchunk*C] with chunk-major columns."""
    c = w.shape[1]
    return np.ascontiguousarray(
        w.reshape(nchunk, 128, c).transpose(1, 0, 2).reshape(128, nchunk * c)
    )


def make_inmaps(inputs: dict):
    xq = np.asarray(inputs["xq"], np.float32)
    xk = np.asarray(inputs["xk"], np.float32)
    xv = np.asarray(inputs["xv"], np.float32)
    wq = np.asarray(inputs["wq"], np.float32)
    bq = np.asarray(inputs["bq"], np.float32)
    wk = np.asarray(inputs["wk"], np.float32)
    bk = np.asarray(inputs["bk"], np.float32)
    wv = np.asarray(inputs["wv"], np.float32)
    bv = np.asarray(inputs["bv"], np.float32)
    wo = np.asarray(inputs["wo"], np.float32)
    valid_lens = np.asarray(inputs["valid_lens"], np.int64)

    nblks = tuple(
        int(min(NST, max(1, -(-int(valid_lens[b]) // 128)))) for b in range(B)
    )
    vrems = tuple(
        min(128, int(valid_lens[b]) - (nblks[b] - 1) * 128) for b in range(B)
    )
    has_bqk = bool(np.any(bq != 0) or np.any(bk != 0))
    has_bv = bool(np.any(bv != 0))

    # shared per-batch transposed activations (bf16)
    xts = {}
    for b in range(B):
        for n, a in (("q", xq), ("k", xk), ("v", xv)):
            xts[f"x{n}t{b}"] = np.ascontiguousarray(a[b].T).astype(npbf16)

    need_mb = any(vrems[b] < 128 for b in range(B))
    if need_mb:
        mb = np.zeros((128, B), np.float32)
        for b in range(B):
            mb[vrems[b] :, b] = -30000.0

    in_maps = []
    for c in range(NCORES):
        sl = slice(c * PW, (c + 1) * PW)
        m = {
            **xts,
            "wq": _chunk_rows(wq[:, sl] * SCALE, NKC).astype(npbf16),
            "wk": _chunk_rows(wk[:, sl], NKC).astype(npbf16),
            "wv": _chunk_rows(wv[:, sl], NKC).astype(npbf16),
            "wo": np.ascontiguousarray(wo[sl, :]).astype(npbf16),
        }
        if need_mb:
            m["mb"] = mb
        if has_bqk:
            m["bqk"] = np.ascontiguousarray(
                np.stack([bq[sl] * SCALE, bk[sl]], axis=1)
            ).astype(npbf16)
        if has_bv:
            m["bvb"] = np.ascontiguousarray(
                np.broadcast_to(bv[sl][None, :], (128, 2 * DH))
            ).astype(npbf16)
        in_maps.append(m)
    return in_maps, nblks, vrems, has_bqk, has_bv


def assemble(results, inputs) -> np.ndarray:
    bo = np.asarray(inputs["bo"], np.float32)
    out = np.zeros((B, S, D), np.float32)
    for c in range(NCORES):
        part = np.asarray(results[c]["outt"], np.float32).reshape(B, D, S)
        for b in range(B):
            out[b] += part[b].T
    out += bo[None, None, :]
    return out


def kernel(**inputs) -> np.ndarray:
    in_maps, nblks, vrems, has_bqk, has_bv = make_inmaps(inputs)
    nc = build_nc(nblks, vrems, has_bqk, has_bv)
    res = run_bass_kernel_spmd(nc, in_maps, core_ids=list(range(NCORES)))
    return assemble(res.results, inputs)


if __name__ == "__main__":
    import reference

    inputs = reference.setup_inputs()
    out = kernel(**{k: np.asarray(v) for k, v in inputs.items()})
    exp = np.asarray(reference.reference(**inputs))
    err = np.linalg.norm(out - exp) / np.linalg.norm(exp)
    print("Relative error:", err)
